# revision 13
# baseline (speedup 1.0000x reference)
"""Despawn2D (8-level db-style DWT analysis + synthesis) on 8 Trainium2 cores.

Math: the reference's FFT circular convolutions with 4-tap filters reduce to
4-tap circular stencils (L = 8192 is a power of two, so the ReplicationPad is
a no-op).  Per level:

  analysis:  out[j]  = f0*a[2j] + f1*a[2j-1] + f2*a[2j-2] + f3*a[2j-3] (mod N)
             with f = h (approx) and f = g (detail), g = flip(h)*(+,-,+,-)

When the provided filter bank is orthogonal (it is for the db2 filters the
reference uses), synthesis(analysis(x)) == x exactly, so the "rec" output is
produced by a DMA copy of the input tile and only the analysis runs on the
compute engines.  A host-side fp64 check of the perfect-reconstruction
property on a small probe vector selects that fast path; otherwise a full
on-device synthesis variant is used.

Fast-path engine split (per core, 2 tiles of 128 rows x 8192):
  - approx chain stays fp32: tap0 on ScalarE, taps 1-3 as in-place fused
    MACs on VectorE (the precision backbone: deep-level coefficients see
    only fp32 rounding).
  - detail chains of levels 0-4 run on the tensor engine in bf16: 4 diag
    matmuls (one per tap, strided rhs reads) accumulating in PSUM, then a
    ScalarE PSUM->SBUF evacuation.  bf16 touches each detail output once
    (~0.2% worst-case), far inside the 2e-2 gate, and bf16 matmul streams
    4x faster than fp32 (1 cyc/col).
  - levels 5-7 (M<=128) stay on ScalarE/VectorE in fp32; their outputs and
    the final approx share one staging tile -> single DMA.
  - input casts x/a_lev -> bf16 split between VectorE (2 elem/cyc) and
    ScalarE; circular-halo copies on GpSimd.
"""

import numpy as np

LEVELS = 8
L = 8192
ROWS_TOTAL = 2048
N_CORES = 8
RPC = ROWS_TOTAL // N_CORES  # rows per core
P = 128  # SBUF partitions
NT = RPC // P  # tiles per core
Nh = L // 2
PE_LEV = 5  # levels [0, PE_LEV) detail chains on the tensor engine

# detail block offsets inside a coeffs row: [d0 | d1 | ... | d7 | a8]
DOFF = []
_off = 0
for _lev in range(LEVELS):
    DOFF.append(_off)
    _off += L >> (_lev + 1)
AOFF = _off  # 8160

_nc_cache = {}


def _make_g(h):
    g = h[::-1].copy()
    g[1::2] *= -1.0
    return g


def _taps_array(scaling):
    """(LEVELS*8,) row: per level [h0..h3, g0..g3], tiled to (P, LEVELS*8)."""
    row = np.empty(LEVELS * 8, np.float32)
    for lev in range(LEVELS):
        h = scaling[lev].astype(np.float32)
        g = _make_g(h)
        row[lev * 8: lev * 8 + 4] = h
        row[lev * 8 + 4: lev * 8 + 8] = g
    return np.tile(row, (P, 1)).copy()


def _pr_is_identity(scaling):
    """fp64 host check: does synthesis(analysis(x)) == x for these filters?"""
    rng = np.random.default_rng(1234)
    n0 = 1 << (LEVELS + 2)
    x = rng.standard_normal((2, n0))
    a = x.copy()
    details = []
    for lev in range(LEVELS):
        h = scaling[lev].astype(np.float64)
        g = _make_g(h)
        N = a.shape[1]
        idx = (np.arange(N // 2)[:, None] * 2 - np.arange(4)[None, :]) % N
        d = (a[:, idx] * g).sum(-1)
        a = (a[:, idx] * h).sum(-1)
        details.append(d)
    r = a
    for lev in reversed(range(LEVELS)):
        h = scaling[lev].astype(np.float64)
        g = _make_g(h)
        d = details[lev]
        m = r.shape[1]
        out = np.empty((2, 2 * m))
        i = np.arange(m)
        out[:, 0::2] = (g[0] * d[:, i] + g[2] * d[:, (i + 1) % m]
                        + h[0] * r[:, i] + h[2] * r[:, (i + 1) % m])
        out[:, 1::2] = (g[1] * d[:, (i + 1) % m] + g[3] * d[:, (i + 2) % m]
                        + h[1] * r[:, (i + 1) % m] + h[3] * r[:, (i + 2) % m])
        r = out
    err = np.abs(r - x).max() / max(np.abs(x).max(), 1e-30)
    return err < 1e-6


def _build_fast():
    """Analysis-only kernel (orthogonal filter bank: rec is a DMA copy).

    All 8 levels run both chains on the tensor engine in fp16 (eps 2^-11):
    per 512-col chunk, diag-matmul taps accumulate in PSUM fp32 (4 approx
    taps, 3 detail taps), then ScalarE evacuates the approx into the next
    level's fp16 ext while VectorE fuses detail tap 3 with its evacuation.
    Details of levels 0-3 stage in fp16 and a SWDGE cast-DMA widens them
    to fp32 in DRAM; levels 4-7 + final approx land in one fp32 tail tile
    -> single DMA.  Tile-0's deep levels are emitted interleaved with
    tile-1's big levels so the tail DMAs issue mid-kernel and the span is
    bounded by the DMA stream, not a compute tail.
    """
    import concourse.bacc as bacc
    import concourse.mybir as mybir
    from concourse.tile import TileContext

    f32 = mybir.dt.float32
    f16 = mybir.dt.float16
    Alu = mybir.AluOpType

    nc = bacc.Bacc()
    x = nc.dram_tensor("x", [RPC, L], f32, kind="ExternalInput")
    taps = nc.dram_tensor("taps", [P, LEVELS * 8], f32, kind="ExternalInput")
    ident = nc.dram_tensor("ident", [P, P], f32, kind="ExternalInput")
    rec = nc.dram_tensor("rec", [RPC, L], f32, kind="ExternalOutput")
    coeffs = nc.dram_tensor("coeffs", [RPC, L], f32, kind="ExternalOutput")

    with TileContext(nc) as tc:
        import contextlib
        with contextlib.ExitStack() as ctx:
            cpool = ctx.enter_context(tc.tile_pool(name="consts", bufs=1))
            xpool = ctx.enter_context(tc.tile_pool(name="xio", bufs=2))
            xfpool = ctx.enter_context(tc.tile_pool(name="xf16", bufs=2))
            apool = ctx.enter_context(tc.tile_pool(name="awork", bufs=2))
            dpool = ctx.enter_context(tc.tile_pool(name="dwork", bufs=2))
            ppool = ctx.enter_context(
                tc.tile_pool(name="psum", bufs=2, space="PSUM"))

            tp = cpool.tile([P, LEVELS * 8], f32)
            nc.sync.dma_start(out=tp[:, :], in_=taps[:, :])
            id_t = cpool.tile([P, P], f32)
            nc.sync.dma_start(out=id_t[:, :], in_=ident[:, :])

            def tap(lev, k):  # h taps
                c = lev * 8 + k
                return tp[:, c:c + 1]

            def gtap(lev, k):  # g taps
                c = lev * 8 + 4 + k
                return tp[:, c:c + 1]

            # fp16 diag weights diag(v) = v * I, built lazily per level so
            # early evacuations are not queued behind the whole bank
            wa = {}
            wd = {}

            def build_weights(lev):
                for k in range(4):
                    wt = cpool.tile([P, P], f16, tag=f"wa{lev}{k}")
                    nc.scalar.mul(wt[:, :], id_t[:, :], tap(lev, k))
                    wa[(lev, k)] = wt
                for k in range(3):
                    wt = cpool.tile([P, P], f16, tag=f"wd{lev}{k}")
                    nc.scalar.mul(wt[:, :], id_t[:, :], gtap(lev, k))
                    wd[(lev, k)] = wt

            # per-tile state
            a16 = [None] * NT
            a32 = [None] * NT
            tail_stage = [None] * NT
            psum_ctr = [0]

            # ---- input loads (1 MiB quarters) + fp16 casts per quarter;
            # rec passthrough queued after all loads.  The wrap halo only
            # feeds a tiny 3-col cast, so the main casts are not gated on
            # the last quarter. ----
            Q = Nh // 2
            xts = [None] * NT
            for t in range(NT):
                rows = slice(t * P, (t + 1) * P)
                xlo = xpool.tile([P, 3 + Nh], f32, tag="xlo")
                xhi = xpool.tile([P, 3 + Nh], f32, tag="xhi")
                # xlo[3+i] = x[i]; xhi[i] = x[Nh-3+i].  Tile 0's first
                # quarter is split so the first matmul starts one
                # half-quarter earlier.
                H = Q // 2
                if t == 0:
                    nc.sync.dma_start(out=xlo[:, 3:3 + H], in_=x[rows, 0:H])
                    nc.sync.dma_start(out=xlo[:, 3 + H:3 + Q],
                                      in_=x[rows, H:Q])
                else:
                    nc.sync.dma_start(out=xlo[:, 3:3 + Q], in_=x[rows, 0:Q])
                nc.sync.dma_start(out=xlo[:, 3 + Q:3 + Nh], in_=x[rows, Q:Nh])
                nc.sync.dma_start(out=xhi[:, 0:3 + Q],
                                  in_=x[rows, Nh - 3:Nh + Q])
                nc.sync.dma_start(out=xhi[:, 3 + Q:3 + Nh],
                                  in_=x[rows, Nh + Q:L])
                xts[t] = (xlo, xhi)
                xf = xfpool.tile([P, 3 + L], f16, tag="xf")
                if t == 0:
                    nc.vector.tensor_copy(out=xf[:, 3:3 + H],
                                          in_=xlo[:, 3:3 + H])
                    nc.vector.tensor_copy(out=xf[:, 3 + H:3 + Q],
                                          in_=xlo[:, 3 + H:3 + Q])
                else:
                    nc.vector.tensor_copy(out=xf[:, 3:3 + Q],
                                          in_=xlo[:, 3:3 + Q])
                nc.vector.tensor_copy(out=xf[:, 3 + Q:3 + Nh],
                                      in_=xlo[:, 3 + Q:3 + Nh])
                nc.vector.tensor_copy(out=xf[:, 3 + Nh:3 + Nh + Q],
                                      in_=xhi[:, 3:3 + Q])
                nc.vector.tensor_copy(out=xf[:, 3 + Nh + Q:3 + L],
                                      in_=xhi[:, 3 + Q:3 + Nh])
                # circular wrap halo: ext[0:3] = x[L-3:L]
                nc.vector.tensor_copy(out=xf[:, 0:3], in_=xhi[:, Nh:Nh + 3])
                a16[t] = xf
            for t in range(NT):
                rows = slice(t * P, (t + 1) * P)
                xlo, xhi = xts[t]
                nc.sync.dma_start(out=rec[rows, 0:Q], in_=xlo[:, 3:3 + Q])
                nc.sync.dma_start(out=rec[rows, Q:Nh], in_=xlo[:, 3 + Q:3 + Nh])
                nc.sync.dma_start(out=rec[rows, Nh:Nh + Q], in_=xhi[:, 3:3 + Q])
                nc.sync.dma_start(out=rec[rows, Nh + Q:L],
                                  in_=xhi[:, 3 + Q:3 + Nh])

            def do_level(lev, t):
                rows = slice(t * P, (t + 1) * P)
                N = L >> lev
                M = N >> 1
                last = False
                if t == 0:
                    build_weights(lev)
                src = a16[t]
                if lev == PE_LEV - 1 and tail_stage[t] is None:
                    tail_stage[t] = dpool.tile([P, 512], f32, tag="tail",
                                               name="tail")
                # approx destination (fp32 at lev4: feeds the fp32 tail)
                if lev < PE_LEV - 1:
                    atag = "a_ev" if lev % 2 == 0 else "a_od"
                    asz = 4099 if lev % 2 == 0 else 2051
                    a_full = apool.tile([P, asz], f16, tag=atag)
                    a_out = a_full[:, 0:M + 3]
                else:
                    a_full = apool.tile([P, 259], f32, tag="a4", name="a4")
                    a_out = a_full[:, 0:M + 3]
                # detail destination
                if lev < PE_LEV - 1:
                    dtag = "d_ev" if lev % 2 == 0 else "d_od"
                    dsz = 4096 if lev % 2 == 0 else 2048
                    d_full = dpool.tile([P, dsz], f16, tag=dtag)
                    dst = d_full[:, 0:M]
                else:
                    col = DOFF[lev] - DOFF[PE_LEV - 1]
                    dst = tail_stage[t][:, col:col + M]

                c_starts = list(range(512, M, 512)) + [0]
                for c0 in c_starts:
                    F = min(512, M - c0)
                    ps_a = ppool.tile([P, 512], f32,
                                      tag=f"pp{psum_ctr[0] % 4}")
                    psum_ctr[0] += 1
                    ps_d = ppool.tile([P, 512], f32,
                                      tag=f"pp{psum_ctr[0] % 4}")
                    psum_ctr[0] += 1
                    for k in range(4):
                        off = (3 - k) + 2 * c0
                        nc.tensor.matmul(
                            ps_a[:, 0:F], wa[(lev, k)],
                            src[:, off:off + 2 * F:2],
                            start=(k == 0), stop=(k == 3))
                    for k in range(3):
                        off = (3 - k) + 2 * c0
                        nc.tensor.matmul(
                            ps_d[:, 0:F], wd[(lev, k)],
                            src[:, off:off + 2 * F:2],
                            start=(k == 0), stop=(k == 2))
                    # approx: plain evacuation on ScalarE; detail: tap 3
                    # fused with the PSUM evacuation on VectorE
                    nc.scalar.copy(
                        out=a_out[:, 3 + c0:3 + c0 + F],
                        in_=ps_a[:, 0:F])
                    nc.vector.scalar_tensor_tensor(
                        out=dst[:, c0:c0 + F],
                        in0=src[:, 2 * c0:2 * c0 + 2 * F:2],
                        scalar=gtap(lev, 3), in1=ps_d[:, 0:F],
                        op0=Alu.mult, op1=Alu.add)
                # circular halo: ext[0:3] = a[M-3:M]
                nc.vector.tensor_copy(
                    out=a_out[:, 0:3], in_=a_out[:, 3 + M - 3:3 + M])
                if lev < PE_LEV - 1:
                    a16[t] = a_full
                    # fp16 staging -> fp32 DRAM cast-DMA
                    nc.gpsimd.dma_start(
                        out=coeffs[rows, DOFF[lev]:DOFF[lev] + M],
                        in_=dst[:, 0:M])
                else:
                    a32[t] = a_full

            def do_tail(t):
                """Levels 5-7 + final approx in fp32 on ScalarE/VectorE
                (M <= 128: matmul would be LDWEIGHTS-paced here)."""
                rows = slice(t * P, (t + 1) * P)
                stage = tail_stage[t]
                src = a32[t]
                for lev in range(PE_LEV, LEVELS):
                    N = L >> lev
                    M = N >> 1
                    last = lev == LEVELS - 1
                    col = DOFF[lev] - DOFF[PE_LEV - 1]
                    dm = stage[:, col:col + M]
                    nc.scalar.mul(dm, src[:, 3:3 + N:2], gtap(lev, 0))
                    for k in (1, 2, 3):
                        nc.vector.scalar_tensor_tensor(
                            out=dm, in0=src[:, 3 - k:3 - k + N:2],
                            scalar=gtap(lev, k), in1=dm,
                            op0=Alu.mult, op1=Alu.add)
                    if not last:
                        a_out = apool.tile([P, M + 3], f32, tag=f"a{lev}",
                                           name=f"a{lev}")
                        am = a_out[:, 3:3 + M]
                    else:
                        am = stage[:, 480:512]
                    nc.scalar.mul(am, src[:, 3:3 + N:2], tap(lev, 0))
                    for k in (1, 2, 3):
                        nc.vector.scalar_tensor_tensor(
                            out=am, in0=src[:, 3 - k:3 - k + N:2],
                            scalar=tap(lev, k), in1=am,
                            op0=Alu.mult, op1=Alu.add)
                    if not last:
                        nc.vector.tensor_copy(
                            out=a_out[:, 0:3], in_=a_out[:, 3 + M - 3:3 + M])
                        src = a_out
                nc.sync.dma_start(
                    out=coeffs[rows, DOFF[PE_LEV - 1]:L],
                    in_=stage[:, 0:512])

            # tile-0 deep levels interleave with tile-1 big levels so the
            # tail DMAs issue mid-kernel
            order = [(0, 0), (1, 0), (2, 0), (3, 0), (0, 1), (4, 0),
                     (1, 1), ("T", 0), (2, 1), (3, 1), (4, 1), ("T", 1)]
            if NT == 1:
                order = [(lev, 0) for lev in range(PE_LEV)] + [("T", 0)]
            for lev, t in order:
                if lev == "T":
                    do_tail(t)
                else:
                    do_level(lev, t)

    nc.finalize()
    return nc


def _build_synth():
    """Full analysis+synthesis fallback for non-orthogonal filter banks
    (unchanged from the validated baseline)."""
    import concourse.bacc as bacc
    import concourse.mybir as mybir
    from concourse.tile import TileContext

    f32 = mybir.dt.float32
    Alu = mybir.AluOpType

    nc = bacc.Bacc()
    x = nc.dram_tensor("x", [RPC, L], f32, kind="ExternalInput")
    taps = nc.dram_tensor("taps", [P, LEVELS * 8], f32, kind="ExternalInput")
    rec = nc.dram_tensor("rec", [RPC, L], f32, kind="ExternalOutput")
    coeffs = nc.dram_tensor("coeffs", [RPC, L], f32, kind="ExternalOutput")

    with TileContext(nc) as tc:
        import contextlib
        with contextlib.ExitStack() as ctx:
            cpool = ctx.enter_context(tc.tile_pool(name="consts", bufs=1))
            xpool = ctx.enter_context(tc.tile_pool(name="xio", bufs=1))
            wpool = ctx.enter_context(tc.tile_pool(name="work", bufs=1))
            dpool = ctx.enter_context(tc.tile_pool(name="dwork", bufs=1))

            tp = cpool.tile([P, LEVELS * 8], f32)
            nc.sync.dma_start(out=tp[:, :], in_=taps[:, :])

            def tap(lev, k):
                c = lev * 8 + k
                return tp[:, c:c + 1]

            def gtap(lev, k):
                c = lev * 8 + 4 + k
                return tp[:, c:c + 1]

            Mh = Nh // 2
            xts = []
            for t in range(NT):
                rows = slice(t * P, (t + 1) * P)
                xlo = xpool.tile([P, 3 + Nh], f32, tag="xlo")
                xhi = xpool.tile([P, 3 + Nh], f32, tag="xhi")
                nc.sync.dma_start(out=xhi[:, 0:3 + Nh], in_=x[rows, Nh - 3:L])
                nc.sync.dma_start(out=xlo[:, 3:3 + Nh], in_=x[rows, 0:Nh])
                nc.vector.tensor_copy(out=xlo[:, 0:3], in_=xhi[:, Nh:Nh + 3])
                xts.append((xlo, xhi))

            a_exts = list(xts)
            d_tiles_all = [[] for _ in range(NT)]
            a_lasts = [None] * NT
            order = [(lev, t) for t in range(NT) for lev in range(LEVELS)]
            for lev, t in order:
                rows = slice(t * P, (t + 1) * P)
                N = L >> lev
                M = N >> 1
                last = lev == LEVELS - 1
                if lev == 0:
                    halves = ((0, xts[t][0], Nh), (Mh, xts[t][1], Nh))
                else:
                    halves = ((0, a_exts[t], N),)
                if not last:
                    a_t = wpool.tile([P, M + 3], f32, tag=f"a{lev}")
                    a_main = a_t[:, 3:3 + M]
                else:
                    a_t = wpool.tile([P, M + 2], f32, tag=f"a{lev}")
                    a_main = a_t[:, 0:M]
                d_t = dpool.tile([P, M + 2], f32, tag=f"d{lev}")
                d_main = d_t[:, 0:M]

                for jb, src, W in halves:
                    W2 = W >> 1
                    am = a_main[:, jb:jb + W2]
                    nc.scalar.mul(am, src[:, 3:3 + W:2], tap(lev, 0))
                    for k in (1, 2, 3):
                        nc.vector.scalar_tensor_tensor(
                            out=am, in0=src[:, 3 - k:3 - k + W:2],
                            scalar=tap(lev, k), in1=am,
                            op0=Alu.mult, op1=Alu.add)
                for jb, src, W in halves:
                    W2 = W >> 1
                    dm = d_main[:, jb:jb + W2]
                    nc.scalar.mul(dm, src[:, 3:3 + W:2], gtap(lev, 0))
                    for k in (1, 2, 3):
                        nc.vector.scalar_tensor_tensor(
                            out=dm, in0=src[:, 3 - k:3 - k + W:2],
                            scalar=gtap(lev, k), in1=dm,
                            op0=Alu.mult, op1=Alu.add)

                nc.sync.dma_start(
                    out=coeffs[rows, DOFF[lev]:DOFF[lev] + M], in_=d_main)
                if last:
                    nc.sync.dma_start(
                        out=coeffs[rows, AOFF:AOFF + M], in_=a_main)

                if not last:
                    nc.vector.tensor_copy(
                        out=a_t[:, 0:3], in_=a_t[:, M:M + 3])
                else:
                    nc.vector.tensor_copy(
                        out=a_t[:, M:M + 2], in_=a_t[:, 0:2])
                d_tiles_all[t].append(d_t)
                a_exts[t] = a_t
                if last:
                    a_lasts[t] = a_t

            # ---------------- synthesis ----------------
            for t in range(NT):
                rows = slice(t * P, (t + 1) * P)
                xlo, xhi = xts[t]
                d_tiles = d_tiles_all[t]
                r_ext = a_lasts[t]
                for lev in reversed(range(LEVELS)):
                    m = L >> (lev + 1)
                    d_t = d_tiles[lev]
                    nc.vector.tensor_copy(
                        out=d_t[:, m:m + 2], in_=d_t[:, 0:2])
                    h4 = [tap(lev, k) for k in range(4)]
                    g4 = [gtap(lev, k) for k in range(4)]
                    if lev > 0:
                        o_t = wpool.tile([P, 2 * m + 2], f32, tag=f"r{lev}")
                        parts = ((0, m, o_t[:, 0:2 * m:2], o_t[:, 1:2 * m:2]),)
                    else:
                        mh = m // 2
                        parts = (
                            (0, mh, xlo[:, 3:3 + Nh:2], xlo[:, 4:3 + Nh:2]),
                            (mh, mh, xhi[:, 3:3 + Nh:2], xhi[:, 4:3 + Nh:2]),
                        )
                    for ib, w, ev, od in parts:
                        nc.vector.tensor_scalar_mul(
                            ev, d_t[:, ib:ib + w], g4[0])
                        for src, s in (
                                (d_t[:, ib + 1:ib + w + 1], g4[2]),
                                (r_ext[:, ib:ib + w], h4[0]),
                                (r_ext[:, ib + 1:ib + w + 1], h4[2])):
                            nc.vector.scalar_tensor_tensor(
                                out=ev, in0=src, scalar=s, in1=ev,
                                op0=Alu.mult, op1=Alu.add)
                        nc.vector.tensor_scalar_mul(
                            od, d_t[:, ib + 1:ib + w + 1], g4[1])
                        for src, s in (
                                (d_t[:, ib + 2:ib + w + 2], g4[3]),
                                (r_ext[:, ib + 1:ib + w + 1], h4[1]),
                                (r_ext[:, ib + 2:ib + w + 2], h4[3])):
                            nc.vector.scalar_tensor_tensor(
                                out=od, in0=src, scalar=s, in1=od,
                                op0=Alu.mult, op1=Alu.add)
                    if lev > 0:
                        nc.vector.tensor_copy(
                            out=o_t[:, 2 * m:2 * m + 2], in_=o_t[:, 0:2])
                        r_ext = o_t
                nc.sync.dma_start(out=rec[rows, 0:Nh], in_=xlo[:, 3:3 + Nh])
                nc.sync.dma_start(out=rec[rows, Nh:L], in_=xhi[:, 3:3 + Nh])

    nc.finalize()
    return nc


def _get_nc(synth: bool):
    key = ("synth", synth)
    if key not in _nc_cache:
        _nc_cache[key] = _build_synth() if synth else _build_fast()
    return _nc_cache[key]


def _in_maps(x, scaling, synth):
    taps = _taps_array(scaling)
    if synth:
        return [
            {"x": np.ascontiguousarray(x[i * RPC:(i + 1) * RPC]), "taps": taps}
            for i in range(N_CORES)
        ]
    ident = np.eye(P, dtype=np.float32)
    return [
        {"x": np.ascontiguousarray(x[i * RPC:(i + 1) * RPC]), "taps": taps,
         "ident": ident}
        for i in range(N_CORES)
    ]


def kernel(x: np.ndarray, scaling: np.ndarray):
    from concourse.bass_utils import run_bass_kernel_spmd

    x = np.ascontiguousarray(np.asarray(x, np.float32))
    scaling = np.asarray(scaling, np.float32)
    assert x.shape == (ROWS_TOTAL, L), x.shape
    assert scaling.shape == (LEVELS, 4), scaling.shape

    synth = not _pr_is_identity(scaling)
    nc = _get_nc(synth)
    in_maps = _in_maps(x, scaling, synth)

    res = None
    last_err = None
    for attempt in range(3):
        try:
            res = run_bass_kernel_spmd(
                nc, in_maps, core_ids=list(range(N_CORES)))
            break
        except Exception as e:  # transient NRT device wedge: retry
            last_err = e
    if res is None:
        raise last_err
    outs = res.results
    rec = np.concatenate([outs[i]["rec"] for i in range(N_CORES)], axis=0)
    coeffs = np.concatenate([outs[i]["coeffs"] for i in range(N_CORES)], axis=0)
    return rec, coeffs
